# revision 6
# baseline (speedup 1.0000x reference)
"""MambaLiteBlock fused Trainium2 kernel, SPMD over 8 NeuronCores.

Problem (reference.py):
    B, T, D, K = 4, 2048, 1024, 7;  H = 2048
    res = x
    xn = layernorm(x) * gamma + beta
    u = xn @ in_w + in_b;  g, v = split(u);  g = sigmoid(g)
    v = causal_dwconv(v, dw_w, dw_b) + (assoc_scan(v, sigmoid(log_decay)) @ mix_w + mix_b)
    y = (g * v) @ out_w + out_b
    return res + y

Sharding: 8 cores = 4 batches x 2 column-halves of H.
Core (b, j) computes, for batch b:
  - layernorm of x[b]            (duplicated across the pair, cheap)
  - in-proj:  g columns for half j, v columns for ALL of H
    (the decay scan feeds mix_w whose contraction needs full H)
  - per-channel decay scan over full H (tensor_tensor_scan along time)
  - causal depthwise conv for half j only
  - mix:      scan(v) @ mix_w[:, half j]
  - out-proj partial: (g * v_new)[:, half j] @ out_w[half j, :]
Host sums the two partials per batch and adds out_b + residual.
No inter-core communication: the only cross-half coupling (out-proj
contraction) is reduced on the host.

Precision: matmuls in bf16 with fp32 PSUM accumulation; the decay scan
runs in fp32 internally (tensor_tensor_scan keeps fp32 state). End to
end this lands at ~2e-3 relative error vs the fp32 reference.

Layout on device: "channels/features on partitions, time on the free
axis". x[b] is loaded token-major, layernormed, downcast to bf16 and
PE-transposed into znT [D, T] which feeds every matmul as the moving
operand; all later tensors stay channel-major so the scan and the
depthwise conv are native (both operate along the free/time axis).
Time is processed in TC-token chunks so SBUF holds only one chunk of
activations; the scan carries its state across chunks via `initial`.
"""

import numpy as np
import ml_dtypes

import concourse.bass as bass
import concourse.tile as tile
import concourse.mybir as mybir
from concourse import bacc
from concourse.masks import make_identity
from concourse.bass_utils import run_bass_kernel_spmd

BT, T, D, KCONV = 4, 2048, 1024, 7
H = 2048
HH = H // 2          # columns per core
P = 128
KT = D // P          # 8  contraction tiles for in-proj
MT_V = H // P        # 16 channel tiles of v (full H)
MT_H = HH // P       # 8  channel tiles of the local half
TC = 256             # tokens per chunk
NCHUNK = T // TC     # 8
TPC = TC // P        # 2  token tiles per chunk

F32 = mybir.dt.float32
BF16 = mybir.dt.bfloat16

_CACHED_NC = None


def _build_core_program():
    nc = bacc.Bacc(None)

    x_d = nc.declare_dram_parameter("x", [T, D], F32, isOutput=False)
    wg_d = nc.declare_dram_parameter("wg", [D, HH], BF16, isOutput=False)
    wv_d = nc.declare_dram_parameter("wv", [D, H], BF16, isOutput=False)
    mixw_d = nc.declare_dram_parameter("mixw", [H, HH], BF16, isOutput=False)
    outw_d = nc.declare_dram_parameter("outw", [HH, D], BF16, isOutput=False)
    bg_d = nc.declare_dram_parameter("bg", [P, MT_H], F32, isOutput=False)
    bv_d = nc.declare_dram_parameter("bv", [P, MT_V], F32, isOutput=False)
    db_d = nc.declare_dram_parameter("db", [P, MT_H], F32, isOutput=False)
    decay_d = nc.declare_dram_parameter("decay", [P, MT_V], F32, isOutput=False)
    dww_d = nc.declare_dram_parameter("dww", [P, MT_H * KCONV], F32, isOutput=False)
    y_d = nc.declare_dram_parameter("y", [T, D], F32, isOutput=True)

    with tile.TileContext(nc) as tc:
        _emit(nc, tc, x_d, wg_d, wv_d, mixw_d, outw_d,
              bg_d, bv_d, db_d, decay_d, dww_d, y_d)
    nc.finalize()
    return nc


def _emit(nc, tc, x_d, wg_d, wv_d, mixw_d, outw_d,
          bg_d, bv_d, db_d, decay_d, dww_d, y_d):
    from contextlib import ExitStack
    ctx = ExitStack()
    with ctx:
        const = ctx.enter_context(tc.tile_pool(name="const", bufs=1))
        xpool = ctx.enter_context(tc.tile_pool(name="xp", bufs=3))
        small = ctx.enter_context(tc.tile_pool(name="small", bufs=4))
        zpool = ctx.enter_context(tc.tile_pool(name="zp", bufs=5))
        zntp = ctx.enter_context(tc.tile_pool(name="znt", bufs=2))
        vpool = ctx.enter_context(tc.tile_pool(name="vp", bufs=2))
        spool = ctx.enter_context(tc.tile_pool(name="sp", bufs=2))
        gpool = ctx.enter_context(tc.tile_pool(name="gp", bufs=2))
        dwpool = ctx.enter_context(tc.tile_pool(name="dwp", bufs=2))
        tppool = ctx.enter_context(tc.tile_pool(name="tp", bufs=2, space="PSUM"))
        mmpool = ctx.enter_context(tc.tile_pool(name="mm", bufs=3, space="PSUM"))
        popool = ctx.enter_context(tc.tile_pool(name="po", bufs=2, space="PSUM"))

        # ---- constants ----
        ident = const.tile([P, P], BF16, tag="ident")
        make_identity(nc, ident[:])
        eps = const.tile([P, 1], F32, tag="eps")
        nc.gpsimd.memset(eps[:], 1e-5)

        wg_sb = const.tile([P, KT * HH], BF16, tag="wg")
        for k in range(KT):
            nc.sync.dma_start(wg_sb[:, k * HH:(k + 1) * HH], wg_d[k * P:(k + 1) * P, :])
        wv_sb = const.tile([P, KT * H], BF16, tag="wv")
        for k in range(KT):
            nc.sync.dma_start(wv_sb[:, k * H:(k + 1) * H], wv_d[k * P:(k + 1) * P, :])
        mixw_sb = const.tile([P, MT_V * HH], BF16, tag="mixw")
        for k in range(MT_V):
            nc.sync.dma_start(mixw_sb[:, k * HH:(k + 1) * HH], mixw_d[k * P:(k + 1) * P, :])
        outw_sb = const.tile([P, MT_H * D], BF16, tag="outw")
        for k in range(MT_H):
            nc.sync.dma_start(outw_sb[:, k * D:(k + 1) * D], outw_d[k * P:(k + 1) * P, :])

        bg_sb = const.tile([P, MT_H], F32, tag="bg")
        nc.sync.dma_start(bg_sb[:], bg_d[:])
        bv_sb = const.tile([P, MT_V], F32, tag="bv")
        nc.sync.dma_start(bv_sb[:], bv_d[:])
        db_sb = const.tile([P, MT_H], F32, tag="db")
        nc.sync.dma_start(db_sb[:], db_d[:])
        decay_sb = const.tile([P, MT_V], F32, tag="decay")
        nc.sync.dma_start(decay_sb[:], decay_d[:])
        dww_sb = const.tile([P, MT_H * KCONV], F32, tag="dww")
        nc.sync.dma_start(dww_sb[:], dww_d[:])

        state_sb = const.tile([P, MT_V], F32, tag="state")

        prev_ext = [None] * MT_V    # previous chunk's v tiles (for conv tails)
        s_tiles = [None] * MT_V     # current chunk's scan outputs
        g_tiles = [None] * MT_H     # current chunk's gates (become g*v_new)

        for nb in range(NCHUNK):
            # ---------- layernorm + transpose: znT [D-part, TC-free] ----------
            z_tiles = []
            for ti in range(TPC):
                t0 = nb * TC + ti * P
                xt = xpool.tile([P, D], F32, tag="x")
                nc.sync.dma_start(xt[:], x_d[t0:t0 + P, :])
                bn6 = small.tile([P, 2 * 6], F32, tag="bn6")
                for c in range(2):
                    nc.vector.bn_stats(bn6[:, c * 6:(c + 1) * 6],
                                       xt[:, c * 512:(c + 1) * 512])
                mv = small.tile([P, 2], F32, tag="mv")
                nc.vector.bn_aggr(mv[:], bn6[:].rearrange("p (c s) -> p c s", s=6))
                std = small.tile([P, 1], F32, tag="std")
                nc.scalar.activation(std[:], mv[:, 1:2],
                                     mybir.ActivationFunctionType.Sqrt,
                                     bias=eps[:])
                rstd = small.tile([P, 1], F32, tag="rstd")
                nc.vector.reciprocal(rstd[:], std[:])
                zt = zpool.tile([P, D], BF16, tag="z")
                nc.vector.tensor_scalar(
                    out=zt[:], in0=xt[:], scalar1=mv[:, 0:1], scalar2=rstd[:],
                    op0=mybir.AluOpType.subtract, op1=mybir.AluOpType.mult)
                z_tiles.append(zt)

            znt = []
            for k in range(KT):
                tp = tppool.tile([P, TC], BF16, tag="tp")
                for ti in range(TPC):
                    nc.tensor.transpose(
                        tp[:, ti * P:(ti + 1) * P],
                        z_tiles[ti][:, k * P:(k + 1) * P],
                        ident[:])
                zk = zntp.tile([P, TC], BF16, tag=f"znt{k}")
                nc.scalar.copy(zk[:], tp[:])
                znt.append(zk)

            # ---------- in-proj ----------
            for mh in range(MT_H):   # gate half
                ps = mmpool.tile([P, TC], F32, tag="mm")
                for k in range(KT):
                    nc.tensor.matmul(
                        ps[:], wg_sb[:, k * HH + mh * P: k * HH + (mh + 1) * P],
                        znt[k][:], start=(k == 0), stop=(k == KT - 1))
                gt = gpool.tile([P, TC], BF16, tag=f"g{mh}")
                nc.scalar.activation(gt[:], ps[:],
                                     mybir.ActivationFunctionType.Sigmoid,
                                     bias=bg_sb[:, mh:mh + 1])
                g_tiles[mh] = gt

            for m in range(MT_V):    # v, full H
                ps = mmpool.tile([P, TC], F32, tag="mm")
                for k in range(KT):
                    nc.tensor.matmul(
                        ps[:], wv_sb[:, k * H + m * P: k * H + (m + 1) * P],
                        znt[k][:], start=(k == 0), stop=(k == KT - 1))
                ext = vpool.tile([P, TC + KCONV - 1], BF16, tag=f"v{m}")
                nc.scalar.add(ext[:, KCONV - 1:], ps[:], bv_sb[:, m:m + 1])
                if nb == 0:
                    nc.gpsimd.memset(ext[:, 0:KCONV - 1], 0.0)
                else:
                    nc.vector.tensor_copy(ext[:, 0:KCONV - 1],
                                          prev_ext[m][:, TC:TC + KCONV - 1])

                # ---------- decay scan (chained across chunks) ----------
                st = spool.tile([P, TC], BF16, tag=f"s{m}")
                nc.vector.tensor_tensor_scan(
                    out=st[:],
                    data0=decay_sb[:, m:m + 1].broadcast_to([P, TC]),
                    data1=ext[:, KCONV - 1:],
                    initial=(0.0 if nb == 0 else state_sb[:, m:m + 1]),
                    op0=mybir.AluOpType.mult, op1=mybir.AluOpType.add)
                nc.vector.tensor_copy(state_sb[:, m:m + 1], st[:, TC - 1:TC])
                s_tiles[m] = st
                prev_ext[m] = ext

            # ---------- depthwise causal conv (local half) ----------
            dw_tiles = []
            for mh in range(MT_H):
                ext = prev_ext[mh]
                dw = dwpool.tile([P, TC], BF16, tag=f"dw{mh}")
                nc.vector.tensor_scalar_mul(dw[:], ext[:, 0:TC],
                                            dww_sb[:, mh * KCONV:mh * KCONV + 1])
                for j in range(1, KCONV):
                    nc.vector.scalar_tensor_tensor(
                        out=dw[:], in0=ext[:, j:j + TC],
                        scalar=dww_sb[:, mh * KCONV + j:mh * KCONV + j + 1],
                        in1=dw[:], op0=mybir.AluOpType.mult,
                        op1=mybir.AluOpType.add)
                dw_tiles.append(dw)

            # ---------- mix + v_new + gate ----------
            for mh in range(MT_H):
                ps = mmpool.tile([P, TC], F32, tag="mm")
                for k in range(MT_V):
                    nc.tensor.matmul(
                        ps[:], mixw_sb[:, k * HH + mh * P: k * HH + (mh + 1) * P],
                        s_tiles[k][:], start=(k == 0), stop=(k == MT_V - 1))
                # v_new = (mix + db) + dwconv    (in place into dw tile)
                nc.vector.scalar_tensor_tensor(
                    out=dw_tiles[mh][:], in0=ps[:],
                    scalar=db_sb[:, mh:mh + 1], in1=dw_tiles[mh][:],
                    op0=mybir.AluOpType.add, op1=mybir.AluOpType.add)
                # g *= v_new  (g tile becomes the out-proj lhsT)
                nc.vector.tensor_tensor(
                    out=g_tiles[mh][:], in0=g_tiles[mh][:], in1=dw_tiles[mh][:],
                    op=mybir.AluOpType.mult)

            # ---------- out-proj partial ----------
            for ti in range(TPC):
                for dc in range(2):
                    ps = popool.tile([P, 512], F32, tag="po")
                    for kh in range(MT_H):
                        nc.tensor.matmul(
                            ps[:], g_tiles[kh][:, ti * P:(ti + 1) * P],
                            outw_sb[:, kh * D + dc * 512: kh * D + (dc + 1) * 512],
                            start=(kh == 0), stop=(kh == MT_H - 1))
                    ysb = zpool.tile([P, 512], F32, tag="ysb")
                    nc.scalar.copy(ysb[:], ps[:])
                    t0 = nb * TC + ti * P
                    nc.sync.dma_start(y_d[t0:t0 + P, dc * 512:(dc + 1) * 512], ysb[:])


def _host_prep(inputs):
    x = np.asarray(inputs["x"], np.float32)
    gamma = np.asarray(inputs["norm_gamma"], np.float32)
    beta = np.asarray(inputs["norm_beta"], np.float32)
    in_w = np.asarray(inputs["in_w"], np.float32)
    in_b = np.asarray(inputs["in_b"], np.float32)
    dw_w = np.asarray(inputs["dw_w"], np.float32)
    dw_b = np.asarray(inputs["dw_b"], np.float32)
    log_decay = np.asarray(inputs["log_decay"], np.float32)
    mix_w = np.asarray(inputs["mix_w"], np.float32)
    mix_b = np.asarray(inputs["mix_b"], np.float32)
    out_w = np.asarray(inputs["out_w"], np.float32)

    w_fold = in_w * gamma[:, None]                     # fold gamma
    b_fold = beta @ in_w + in_b                        # fold beta
    decay = 1.0 / (1.0 + np.exp(-log_decay))
    db = dw_b + mix_b

    bf16 = ml_dtypes.bfloat16
    in_maps = []
    for c in range(8):
        b, j = divmod(c, 2)
        hs = j * HH
        # The device program always treats v-channel tiles 0..MT_H-1 as "the
        # local half" (depthwise conv, mix output, gating, out-proj).  Permute
        # the H axis per core so the local half comes first; everything that
        # indexes full H (wv columns, bv, decay, mixw rows) gets the same
        # permutation.
        perm = np.concatenate([np.arange(hs, hs + HH),
                               np.arange((1 - j) * HH, (1 - j) * HH + HH)])
        m = {
            "x": np.ascontiguousarray(x[b]),
            "wg": np.ascontiguousarray(w_fold[:, hs:hs + HH].astype(bf16)),
            "wv": np.ascontiguousarray(w_fold[:, H + perm].astype(bf16)),
            "mixw": np.ascontiguousarray(mix_w[perm][:, hs:hs + HH].astype(bf16)),
            "outw": np.ascontiguousarray(out_w[hs:hs + HH, :].astype(bf16)),
            "bg": np.ascontiguousarray(
                b_fold[hs:hs + HH].reshape(MT_H, P).T.astype(np.float32)),
            "bv": np.ascontiguousarray(
                b_fold[H + perm].reshape(MT_V, P).T.astype(np.float32)),
            "db": np.ascontiguousarray(
                db[hs:hs + HH].reshape(MT_H, P).T.astype(np.float32)),
            "decay": np.ascontiguousarray(
                decay[perm].reshape(MT_V, P).T.astype(np.float32)),
            "dww": np.ascontiguousarray(
                dw_w[hs:hs + HH].reshape(MT_H, P, KCONV)
                .transpose(1, 0, 2).reshape(P, MT_H * KCONV).astype(np.float32)),
        }
        in_maps.append(m)
    return in_maps


def get_nc():
    global _CACHED_NC
    if _CACHED_NC is None:
        _CACHED_NC = _build_core_program()
    return _CACHED_NC


def kernel(**inputs):
    nc = get_nc()
    in_maps = _host_prep(inputs)
    res = run_bass_kernel_spmd(nc, in_maps, list(range(8)))

    x = np.asarray(inputs["x"], np.float32)
    out_b = np.asarray(inputs["out_b"], np.float32)
    y = np.empty((BT, T, D), np.float32)
    for b in range(BT):
        y[b] = res.results[2 * b]["y"] + res.results[2 * b + 1]["y"]
    y += out_b
    y += x
    return y


# revision 10
# speedup vs baseline: 1.3443x; 1.3443x over previous
"""MambaLiteBlock fused Trainium2 kernel, SPMD over 8 NeuronCores.

Problem (reference.py):
    B, T, D, K = 4, 2048, 1024, 7;  H = 2048
    res = x
    xn = layernorm(x) * gamma + beta
    u = xn @ in_w + in_b;  g, v = split(u);  g = sigmoid(g)
    v = causal_dwconv(v, dw_w, dw_b) + (assoc_scan(v, sigmoid(log_decay)) @ mix_w + mix_b)
    y = (g * v) @ out_w + out_b
    return res + y

Sharding: 8 cores = 4 batches x 2 column-halves of H.
Core (b, j) computes, for batch b:
  - layernorm of x[b]            (duplicated across the pair, cheap)
  - in-proj:  g columns for half j, v columns for ALL of H
    (the decay scan feeds mix_w whose contraction needs full H)
  - per-channel decay scan over full H (tensor_tensor_scan along time)
  - causal depthwise conv for half j only
  - mix:      scan(v) @ mix_w[:, half j]
  - out-proj partial: (g * v_new)[:, half j] @ out_w[half j, :]
Host sums the two partials per batch and adds out_b + residual.
No inter-core communication: the only cross-half coupling (out-proj
contraction) is reduced on the host.

Precision: matmuls in bf16 with fp32 PSUM accumulation; the decay scan
runs in fp32 internally (tensor_tensor_scan keeps fp32 state). End to
end this lands at ~2e-3 relative error vs the fp32 reference.

Layout on device: "channels/features on partitions, time on the free
axis". x[b] is loaded token-major, layernormed, downcast to bf16 and
PE-transposed into znT [D, T] which feeds every matmul as the moving
operand; all later tensors stay channel-major so the scan and the
depthwise conv are native (both operate along the free/time axis).
Time is processed in TC-token chunks so SBUF holds only one chunk of
activations; the scan carries its state across chunks via `initial`.
"""

import numpy as np
import ml_dtypes

import concourse.bass as bass
import concourse.tile as tile
import concourse.mybir as mybir
from concourse import bacc
from concourse.masks import make_identity
from concourse.bass_utils import run_bass_kernel_spmd

BT, T, D, KCONV = 4, 2048, 1024, 7
H = 2048
HH = H // 2          # columns per core
P = 128
KT = D // P          # 8  contraction tiles for in-proj
MT_V = H // P        # 16 channel tiles of v (full H)
MT_H = HH // P       # 8  channel tiles of the local half
TC = 256             # tokens per chunk
NCHUNK = T // TC     # 8
TPC = TC // P        # 2  token tiles per chunk

F32 = mybir.dt.float32
BF16 = mybir.dt.bfloat16

_CACHED_NC = None


def _build_core_program():
    nc = bacc.Bacc(None)

    x_d = nc.declare_dram_parameter("x", [T, D], F32, isOutput=False)
    wg_d = nc.declare_dram_parameter("wg", [D, HH], BF16, isOutput=False)
    wv_d = nc.declare_dram_parameter("wv", [D, H], BF16, isOutput=False)
    mixw_d = nc.declare_dram_parameter("mixw", [H, HH], BF16, isOutput=False)
    outw_d = nc.declare_dram_parameter("outw", [HH, D], BF16, isOutput=False)
    bg_d = nc.declare_dram_parameter("bg", [P, MT_H], F32, isOutput=False)
    bv_d = nc.declare_dram_parameter("bv", [P, MT_V], F32, isOutput=False)
    db_d = nc.declare_dram_parameter("db", [P, MT_H], F32, isOutput=False)
    decay_d = nc.declare_dram_parameter("decay", [P, MT_V], F32, isOutput=False)
    dww_d = nc.declare_dram_parameter("dww", [P, MT_H * KCONV], F32, isOutput=False)
    y_d = nc.declare_dram_parameter("y", [T, D], F32, isOutput=True)

    with tile.TileContext(nc) as tc:
        _emit(nc, tc, x_d, wg_d, wv_d, mixw_d, outw_d,
              bg_d, bv_d, db_d, decay_d, dww_d, y_d)
    nc.finalize()
    return nc


def _emit(nc, tc, x_d, wg_d, wv_d, mixw_d, outw_d,
          bg_d, bv_d, db_d, decay_d, dww_d, y_d):
    from contextlib import ExitStack
    ctx = ExitStack()
    with ctx:
        const = ctx.enter_context(tc.tile_pool(name="const", bufs=1))
        xpool = ctx.enter_context(tc.tile_pool(name="xp", bufs=3))
        small = ctx.enter_context(tc.tile_pool(name="small", bufs=4))
        zpool = ctx.enter_context(tc.tile_pool(name="zp", bufs=5))
        zntp = ctx.enter_context(tc.tile_pool(name="znt", bufs=2))
        vpool = ctx.enter_context(tc.tile_pool(name="vp", bufs=2))
        spool = ctx.enter_context(tc.tile_pool(name="sp", bufs=2))
        gpool = ctx.enter_context(tc.tile_pool(name="gp", bufs=2))
        dwpool = ctx.enter_context(tc.tile_pool(name="dwp", bufs=2))
        tppool = ctx.enter_context(tc.tile_pool(name="tp", bufs=2, space="PSUM"))
        mmpool = ctx.enter_context(tc.tile_pool(name="mm", bufs=3, space="PSUM"))
        popool = ctx.enter_context(tc.tile_pool(name="po", bufs=2, space="PSUM"))

        # ---- constants ----
        ident = const.tile([P, P], BF16, tag="ident")
        make_identity(nc, ident[:])
        eps = const.tile([P, 1], F32, tag="eps")
        nc.gpsimd.memset(eps[:], 1e-5)

        wg_sb = const.tile([P, KT * HH], BF16, tag="wg")
        for k in range(KT):
            nc.sync.dma_start(wg_sb[:, k * HH:(k + 1) * HH], wg_d[k * P:(k + 1) * P, :])
        wv_sb = const.tile([P, KT * H], BF16, tag="wv")
        for k in range(KT):
            nc.sync.dma_start(wv_sb[:, k * H:(k + 1) * H], wv_d[k * P:(k + 1) * P, :])
        mixw_sb = const.tile([P, MT_V * HH], BF16, tag="mixw")
        for k in range(MT_V):
            nc.sync.dma_start(mixw_sb[:, k * HH:(k + 1) * HH], mixw_d[k * P:(k + 1) * P, :])
        outw_sb = const.tile([P, MT_H * D], BF16, tag="outw")
        for k in range(MT_H):
            nc.sync.dma_start(outw_sb[:, k * D:(k + 1) * D], outw_d[k * P:(k + 1) * P, :])

        bg_sb = const.tile([P, MT_H], F32, tag="bg")
        nc.sync.dma_start(bg_sb[:], bg_d[:])
        bv_sb = const.tile([P, MT_V], F32, tag="bv")
        nc.sync.dma_start(bv_sb[:], bv_d[:])
        db_sb = const.tile([P, MT_H], F32, tag="db")
        nc.sync.dma_start(db_sb[:], db_d[:])
        decay_sb = const.tile([P, MT_V], F32, tag="decay")
        nc.sync.dma_start(decay_sb[:], decay_d[:])
        dww_sb = const.tile([P, MT_H * KCONV], F32, tag="dww")
        nc.sync.dma_start(dww_sb[:], dww_d[:])

        state_sb = const.tile([P, MT_V], F32, tag="state")

        prev_ext = [None] * MT_V    # previous chunk's v tiles (for conv tails)
        s_tiles = [None] * MT_V     # current chunk's scan outputs
        g_tiles = [None] * MT_H     # current chunk's gates (become g*v_new)

        for nb in range(NCHUNK):
            # ---------- layernorm + transpose: znT [D-part, TC-free] ----------
            z_tiles = []
            for ti in range(TPC):
                t0 = nb * TC + ti * P
                xt = xpool.tile([P, D], F32, tag="x")
                nc.sync.dma_start(xt[:], x_d[t0:t0 + P, :])
                bn6 = small.tile([P, 2 * 6], F32, tag="bn6")
                for c in range(2):
                    nc.vector.bn_stats(bn6[:, c * 6:(c + 1) * 6],
                                       xt[:, c * 512:(c + 1) * 512])
                mv = small.tile([P, 2], F32, tag="mv")
                nc.vector.bn_aggr(mv[:], bn6[:].rearrange("p (c s) -> p c s", s=6))
                std = small.tile([P, 1], F32, tag="std")
                nc.scalar.activation(std[:], mv[:, 1:2],
                                     mybir.ActivationFunctionType.Sqrt,
                                     bias=eps[:])
                rstd = small.tile([P, 1], F32, tag="rstd")
                nc.vector.reciprocal(rstd[:], std[:])
                zt = zpool.tile([P, D], BF16, tag="z")
                nc.vector.tensor_scalar(
                    out=zt[:], in0=xt[:], scalar1=mv[:, 0:1], scalar2=rstd[:],
                    op0=mybir.AluOpType.subtract, op1=mybir.AluOpType.mult)
                z_tiles.append(zt)

            znt = []
            for k in range(KT):
                tp = tppool.tile([P, TC], BF16, tag="tp")
                for ti in range(TPC):
                    nc.tensor.transpose(
                        tp[:, ti * P:(ti + 1) * P],
                        z_tiles[ti][:, k * P:(k + 1) * P],
                        ident[:])
                zk = zntp.tile([P, TC], BF16, tag=f"znt{k}")
                nc.scalar.copy(zk[:], tp[:])
                znt.append(zk)

            # ---------- in-proj ----------
            for mh in range(MT_H):   # gate half
                ps = mmpool.tile([P, TC], F32, tag="mm")
                for k in range(KT):
                    nc.tensor.matmul(
                        ps[:], wg_sb[:, k * HH + mh * P: k * HH + (mh + 1) * P],
                        znt[k][:], start=(k == 0), stop=(k == KT - 1))
                gt = gpool.tile([P, TC], BF16, tag=f"g{mh}")
                nc.scalar.activation(gt[:], ps[:],
                                     mybir.ActivationFunctionType.Sigmoid,
                                     bias=bg_sb[:, mh:mh + 1])
                g_tiles[mh] = gt

            for m in range(MT_V):    # v, full H
                ps = mmpool.tile([P, TC], F32, tag="mm")
                for k in range(KT):
                    nc.tensor.matmul(
                        ps[:], wv_sb[:, k * H + m * P: k * H + (m + 1) * P],
                        znt[k][:], start=(k == 0), stop=(k == KT - 1))
                ext = vpool.tile([P, TC + KCONV - 1], BF16, tag=f"v{m}")
                nc.scalar.add(ext[:, KCONV - 1:], ps[:], bv_sb[:, m:m + 1])
                if nb == 0:
                    nc.gpsimd.memset(ext[:, 0:KCONV - 1], 0.0)
                else:
                    nc.vector.tensor_copy(ext[:, 0:KCONV - 1],
                                          prev_ext[m][:, TC:TC + KCONV - 1])

                # ---------- decay scan (chained across chunks) ----------
                st = spool.tile([P, TC], BF16, tag=f"s{m}")
                nc.vector.tensor_tensor_scan(
                    out=st[:],
                    data0=decay_sb[:, m:m + 1].broadcast_to([P, TC]),
                    data1=ext[:, KCONV - 1:],
                    initial=(0.0 if nb == 0 else state_sb[:, m:m + 1]),
                    op0=mybir.AluOpType.mult, op1=mybir.AluOpType.add)
                nc.vector.tensor_copy(state_sb[:, m:m + 1], st[:, TC - 1:TC])
                s_tiles[m] = st
                prev_ext[m] = ext

            # ---------- depthwise causal conv (local half) ----------
            dw_tiles = []
            for mh in range(MT_H):
                ext = prev_ext[mh]
                dw = dwpool.tile([P, TC], BF16, tag=f"dw{mh}")
                nc.vector.tensor_scalar_mul(dw[:], ext[:, 0:TC],
                                            dww_sb[:, mh * KCONV:mh * KCONV + 1])
                for j in range(1, KCONV):
                    nc.vector.scalar_tensor_tensor(
                        out=dw[:], in0=ext[:, j:j + TC],
                        scalar=dww_sb[:, mh * KCONV + j:mh * KCONV + j + 1],
                        in1=dw[:], op0=mybir.AluOpType.mult,
                        op1=mybir.AluOpType.add)
                dw_tiles.append(dw)

            # ---------- mix + v_new + gate ----------
            for mh in range(MT_H):
                ps = mmpool.tile([P, TC], F32, tag="mm")
                for k in range(MT_V):
                    nc.tensor.matmul(
                        ps[:], mixw_sb[:, k * HH + mh * P: k * HH + (mh + 1) * P],
                        s_tiles[k][:], start=(k == 0), stop=(k == MT_V - 1))
                # v_new = (mix + db) + dwconv    (in place into dw tile)
                nc.vector.scalar_tensor_tensor(
                    out=dw_tiles[mh][:], in0=ps[:],
                    scalar=db_sb[:, mh:mh + 1], in1=dw_tiles[mh][:],
                    op0=mybir.AluOpType.add, op1=mybir.AluOpType.add)
                # g *= v_new  (g tile becomes the out-proj lhsT)
                nc.vector.tensor_tensor(
                    out=g_tiles[mh][:], in0=g_tiles[mh][:], in1=dw_tiles[mh][:],
                    op=mybir.AluOpType.mult)

            # ---------- out-proj partial ----------
            for ti in range(TPC):
                for dc in range(2):
                    ps = popool.tile([P, 512], F32, tag="po")
                    for kh in range(MT_H):
                        nc.tensor.matmul(
                            ps[:], g_tiles[kh][:, ti * P:(ti + 1) * P],
                            outw_sb[:, kh * D + dc * 512: kh * D + (dc + 1) * 512],
                            start=(kh == 0), stop=(kh == MT_H - 1))
                    ysb = zpool.tile([P, 512], F32, tag="ysb")
                    nc.scalar.copy(ysb[:], ps[:])
                    t0 = nb * TC + ti * P
                    nc.sync.dma_start(y_d[t0:t0 + P, dc * 512:(dc + 1) * 512], ysb[:])


def _host_prep(inputs):
    x = np.asarray(inputs["x"], np.float32)
    gamma = np.asarray(inputs["norm_gamma"], np.float32)
    beta = np.asarray(inputs["norm_beta"], np.float32)
    in_w = np.asarray(inputs["in_w"], np.float32)
    in_b = np.asarray(inputs["in_b"], np.float32)
    dw_w = np.asarray(inputs["dw_w"], np.float32)
    dw_b = np.asarray(inputs["dw_b"], np.float32)
    log_decay = np.asarray(inputs["log_decay"], np.float32)
    mix_w = np.asarray(inputs["mix_w"], np.float32)
    mix_b = np.asarray(inputs["mix_b"], np.float32)
    out_w = np.asarray(inputs["out_w"], np.float32)

    w_fold = in_w * gamma[:, None]                     # fold gamma
    b_fold = beta @ in_w + in_b                        # fold beta
    decay = 1.0 / (1.0 + np.exp(-log_decay))
    db = dw_b + mix_b

    bf16 = ml_dtypes.bfloat16
    in_maps = []
    for c in range(8):
        b, j = divmod(c, 2)
        hs = j * HH
        # The device program always treats v-channel tiles 0..MT_H-1 as "the
        # local half" (depthwise conv, mix output, gating, out-proj).  Permute
        # the H axis per core so the local half comes first; everything that
        # indexes full H (wv columns, bv, decay, mixw rows) gets the same
        # permutation.
        perm = np.concatenate([np.arange(hs, hs + HH),
                               np.arange((1 - j) * HH, (1 - j) * HH + HH)])
        m = {
            "x": np.ascontiguousarray(x[b]),
            "wg": np.ascontiguousarray(w_fold[:, hs:hs + HH].astype(bf16)),
            "wv": np.ascontiguousarray(w_fold[:, H + perm].astype(bf16)),
            "mixw": np.ascontiguousarray(mix_w[perm][:, hs:hs + HH].astype(bf16)),
            "outw": np.ascontiguousarray(out_w[hs:hs + HH, :].astype(bf16)),
            "bg": np.ascontiguousarray(
                b_fold[hs:hs + HH].reshape(MT_H, P).T.astype(np.float32)),
            "bv": np.ascontiguousarray(
                b_fold[H + perm].reshape(MT_V, P).T.astype(np.float32)),
            "db": np.ascontiguousarray(
                db[hs:hs + HH].reshape(MT_H, P).T.astype(np.float32)),
            "decay": np.ascontiguousarray(
                decay[perm].reshape(MT_V, P).T.astype(np.float32)),
            "dww": np.ascontiguousarray(
                dw_w[hs:hs + HH].reshape(MT_H, P, KCONV)
                .transpose(1, 0, 2).reshape(P, MT_H * KCONV).astype(np.float32)),
        }
        in_maps.append(m)
    return in_maps


def get_nc():
    global _CACHED_NC
    if _CACHED_NC is None:
        _CACHED_NC = _build_core_program()
    return _CACHED_NC


_RUNNER = None


def _get_runner():
    """Build the 8-core shard_map runner once; repeated calls then skip
    re-tracing/compiling (run_bass_via_pjrt builds a fresh jit per call)."""
    global _RUNNER
    if _RUNNER is not None:
        return _RUNNER
    import jax
    from jax.sharding import Mesh, PartitionSpec
    from jax.experimental.shard_map import shard_map
    import concourse.mybir as mb
    from concourse import bass2jax

    nc = get_nc()
    bass2jax.install_neuronx_cc_hook()

    partition_name = (nc.partition_id_tensor.name
                      if nc.partition_id_tensor else None)
    in_names, out_names, out_avals, zero_shapes = [], [], [], []
    for alloc in nc.m.functions[0].allocations:
        if not isinstance(alloc, mb.MemoryLocationSet):
            continue
        name = alloc.memorylocations[0].name
        if alloc.kind == "ExternalInput":
            if name != partition_name:
                in_names.append(name)
        elif alloc.kind == "ExternalOutput":
            out_names.append(name)
            shape = tuple(alloc.tensor_shape)
            dtype = mb.dt.np(alloc.dtype)
            out_avals.append(jax.core.ShapedArray(shape, dtype))
            zero_shapes.append((shape, dtype))
    n_params = len(in_names)
    all_names = in_names + out_names
    if partition_name is not None:
        all_names = all_names + [partition_name]
    donate = tuple(range(n_params, n_params + len(out_names)))

    def _body(*args):
        operands = list(args)
        if partition_name is not None:
            operands.append(bass2jax.partition_id_tensor())
        outs = bass2jax._bass_exec_p.bind(
            *operands,
            out_avals=tuple(out_avals),
            in_names=tuple(all_names),
            out_names=tuple(out_names),
            lowering_input_output_aliases=(),
            sim_require_finite=True,
            sim_require_nnan=True,
            nc=nc,
        )
        return tuple(outs)

    devices = jax.devices()[:8]
    mesh = Mesh(np.asarray(devices), ("core",))
    nio = n_params + len(out_names)
    sharded = jax.jit(
        shard_map(_body, mesh=mesh,
                  in_specs=(PartitionSpec("core"),) * nio,
                  out_specs=(PartitionSpec("core"),) * len(out_names),
                  check_rep=False),
        donate_argnums=donate, keep_unused=True)
    _RUNNER = (sharded, in_names, out_names, out_avals, zero_shapes)
    return _RUNNER


def _run_device(in_maps):
    sharded, in_names, out_names, out_avals, zero_shapes = _get_runner()
    concat_in = [
        np.concatenate([in_maps[c][n] for c in range(8)], axis=0)
        for n in in_names
    ]
    concat_zeros = [np.zeros((8 * s[0], *s[1:]), d) for s, d in zero_shapes]
    out_arrs = sharded(*concat_in, *concat_zeros)
    return [
        {n: np.asarray(out_arrs[i]).reshape(8, *out_avals[i].shape)[c]
         for i, n in enumerate(out_names)}
        for c in range(8)
    ]


def kernel(**inputs):
    in_maps = _host_prep(inputs)
    results = _run_device(in_maps)

    x = np.asarray(inputs["x"], np.float32)
    out_b = np.asarray(inputs["out_b"], np.float32)
    y = np.empty((BT, T, D), np.float32)
    for b in range(BT):
        y[b] = results[2 * b]["y"] + results[2 * b + 1]["y"]
    y += out_b
    y += x
    return y


# revision 37
# speedup vs baseline: 38919.6309x; 28950.6994x over previous
"""MambaLiteBlock fused Trainium2 kernel, SPMD over 8 NeuronCores.

Problem (reference.py):
    B, T, D, K = 4, 2048, 1024, 7;  H = 2048
    res = x
    xn = layernorm(x) * gamma + beta
    u = xn @ in_w + in_b;  g, v = split(u);  g = sigmoid(g)
    v = causal_dwconv(v, dw_w, dw_b) + (assoc_scan(v, sigmoid(log_decay)) @ mix_w + mix_b)
    y = (g * v) @ out_w + out_b
    return res + y

Sharding: 8 cores = 4 batches x 2 column-halves of H.
Core (b, j) computes, for batch b:
  - layernorm of x[b]            (duplicated across the pair, cheap)
  - in-proj:  g columns for half j, v columns for ALL of H
    (the decay scan feeds mix_w whose contraction needs full H)
  - per-channel decay scan over full H (tensor_tensor_scan along time)
  - causal depthwise conv for half j only
  - mix:      scan(v) @ mix_w[:, half j]
  - out-proj partial: (g * v_new)[:, half j] @ out_w[half j, :]
Host sums the two partials per batch and adds out_b + residual.
No inter-core communication: the only cross-half coupling (out-proj
contraction) is reduced on the host.

Precision: matmuls in bf16 with fp32 PSUM accumulation; the decay scan
runs in fp32 internally (tensor_tensor_scan keeps fp32 state). End to
end this lands at ~2e-3 relative error vs the fp32 reference.

Layout on device: "channels/features on partitions, time on the free
axis". x[b] is loaded token-major, layernormed, downcast to bf16 and
PE-transposed into znT [D, T] which feeds every matmul as the moving
operand; all later tensors stay channel-major so the scan and the
depthwise conv are native (both operate along the free/time axis).
Time is processed in TC-token chunks so SBUF holds only one chunk of
activations; the scan carries its state across chunks via `initial`.
"""

import numpy as np
import ml_dtypes

import concourse.bass as bass
import concourse.tile as tile
import concourse.mybir as mybir
from concourse import bacc
from concourse.masks import make_identity
from concourse.bass_utils import run_bass_kernel_spmd

BT, T, D, KCONV = 4, 2048, 1024, 7
H = 2048
HH = H // 2          # columns per core
P = 128
KT = D // P          # 8  contraction tiles for in-proj
MT_V = H // P        # 16 channel tiles of v (full H)
MT_H = HH // P       # 8  channel tiles of the local half
TC = 256             # tokens per chunk
NCHUNK = T // TC     # 8
TPC = TC // P        # 2  token tiles per chunk

F32 = mybir.dt.float32
BF16 = mybir.dt.bfloat16

_CACHED_NC = None


def _build_core_program(reps=1):
    nc = bacc.Bacc(None)

    x_d = nc.declare_dram_parameter("x", [T, D], F32, isOutput=False)
    wg_d = nc.declare_dram_parameter("wg", [D, HH], BF16, isOutput=False)
    wv_d = nc.declare_dram_parameter("wv", [D, H], BF16, isOutput=False)
    mixw_d = nc.declare_dram_parameter("mixw", [H, HH], BF16, isOutput=False)
    outw_d = nc.declare_dram_parameter("outw", [HH, D], BF16, isOutput=False)
    bg_d = nc.declare_dram_parameter("bg", [P, MT_H], F32, isOutput=False)
    bv_d = nc.declare_dram_parameter("bv", [P, MT_V], F32, isOutput=False)
    db_d = nc.declare_dram_parameter("db", [P, MT_H], F32, isOutput=False)
    decay_d = nc.declare_dram_parameter("decay", [P, MT_V], F32, isOutput=False)
    dww_d = nc.declare_dram_parameter("dww", [P, MT_H * KCONV], F32, isOutput=False)
    y_d = nc.declare_dram_parameter("y", [T, D], F32, isOutput=True)

    with tile.TileContext(nc) as tc:
        _emit(nc, tc, x_d, wg_d, wv_d, mixw_d, outw_d,
              bg_d, bv_d, db_d, decay_d, dww_d, y_d, reps=reps)
    nc.finalize()
    return nc


def _emit(nc, tc, x_d, wg_d, wv_d, mixw_d, outw_d,
          bg_d, bv_d, db_d, decay_d, dww_d, y_d, reps=1):
    from contextlib import ExitStack
    ctx = ExitStack()
    with ctx:
        const = ctx.enter_context(tc.tile_pool(name="const", bufs=1))
        xpool = ctx.enter_context(tc.tile_pool(name="xp", bufs=3))
        small = ctx.enter_context(tc.tile_pool(name="small", bufs=4))
        zpool = ctx.enter_context(tc.tile_pool(name="zp", bufs=5))
        zntp = ctx.enter_context(tc.tile_pool(name="znt", bufs=2))
        vpool = ctx.enter_context(tc.tile_pool(name="vp", bufs=2))
        spool = ctx.enter_context(tc.tile_pool(name="sp", bufs=2))
        gpool = ctx.enter_context(tc.tile_pool(name="gp", bufs=2))
        dwpool = ctx.enter_context(tc.tile_pool(name="dwp", bufs=2))
        tppool = ctx.enter_context(tc.tile_pool(name="tp", bufs=2, space="PSUM"))
        mmpool = ctx.enter_context(tc.tile_pool(name="mm", bufs=4, space="PSUM"))
        popool = ctx.enter_context(tc.tile_pool(name="po", bufs=2, space="PSUM"))

        # ---- constants ----
        ident = const.tile([P, P], BF16, tag="ident")
        make_identity(nc, ident[:])
        eps = const.tile([P, 1], F32, tag="eps")
        nc.gpsimd.memset(eps[:], 1e-5)

        wg_sb = const.tile([P, KT * HH], BF16, tag="wg")
        for k in range(KT):
            nc.sync.dma_start(wg_sb[:, k * HH:(k + 1) * HH], wg_d[k * P:(k + 1) * P, :])
        wv_sb = const.tile([P, KT * H], BF16, tag="wv")
        for k in range(KT):
            nc.sync.dma_start(wv_sb[:, k * H:(k + 1) * H], wv_d[k * P:(k + 1) * P, :])
        mixw_sb = const.tile([P, MT_V * HH], BF16, tag="mixw")
        for k in range(MT_V):
            nc.sync.dma_start(mixw_sb[:, k * HH:(k + 1) * HH], mixw_d[k * P:(k + 1) * P, :])
        outw_sb = const.tile([P, MT_H * D], BF16, tag="outw")
        for k in range(MT_H):
            nc.sync.dma_start(outw_sb[:, k * D:(k + 1) * D], outw_d[k * P:(k + 1) * P, :])

        bg_sb = const.tile([P, MT_H], F32, tag="bg")
        nc.sync.dma_start(bg_sb[:], bg_d[:])
        bv_sb = const.tile([P, MT_V], F32, tag="bv")
        nc.sync.dma_start(bv_sb[:], bv_d[:])
        db_sb = const.tile([P, MT_H], F32, tag="db")
        nc.sync.dma_start(db_sb[:], db_d[:])
        decay_sb = const.tile([P, MT_V], F32, tag="decay")
        nc.sync.dma_start(decay_sb[:], decay_d[:])
        dww_sb = const.tile([P, MT_H * KCONV], F32, tag="dww")
        nc.sync.dma_start(dww_sb[:], dww_d[:])

        state_sb = const.tile([P, MT_V], F32, tag="state")

        prev_ext = [None] * MT_V    # previous chunk's v tiles (for conv tails)
        s_tiles = [None] * MT_V     # current chunk's scan outputs
        g_tiles = [None] * MT_H     # current chunk's gates (become g*v_new)

        for _rep, nb in ((r, c) for r in range(reps) for c in range(NCHUNK)):
            # ---------- layernorm + transpose: znT [D-part, TC-free] ----------
            z_tiles = []
            for ti in range(TPC):
                t0 = nb * TC + ti * P
                xt = xpool.tile([P, D], F32, tag="x")
                nc.sync.dma_start(xt[:], x_d[t0:t0 + P, :])
                bn6 = small.tile([P, 2 * 6], F32, tag="bn6")
                for c in range(2):
                    nc.vector.bn_stats(bn6[:, c * 6:(c + 1) * 6],
                                       xt[:, c * 512:(c + 1) * 512])
                mv = small.tile([P, 2], F32, tag="mv")
                nc.vector.bn_aggr(mv[:], bn6[:].rearrange("p (c s) -> p c s", s=6))
                std = small.tile([P, 1], F32, tag="std")
                nc.scalar.activation(std[:], mv[:, 1:2],
                                     mybir.ActivationFunctionType.Sqrt,
                                     bias=eps[:])
                rstd = small.tile([P, 1], F32, tag="rstd")
                nc.vector.reciprocal(rstd[:], std[:])
                # z = (x - mu) * rstd on ACT: Identity(x*rstd + (-mu*rstd))
                nmr = small.tile([P, 1], F32, tag="nmr")
                nc.vector.tensor_scalar(
                    out=nmr[:], in0=mv[:, 0:1], scalar1=rstd[:], scalar2=-1.0,
                    op0=mybir.AluOpType.mult, op1=mybir.AluOpType.mult)
                zt = zpool.tile([P, D], BF16, tag="z")
                nc.scalar.activation(zt[:], xt[:],
                                     mybir.ActivationFunctionType.Identity,
                                     bias=nmr[:], scale=rstd[:])
                z_tiles.append(zt)

            znt = []
            for k in range(KT):
                tp = tppool.tile([P, TC], BF16, tag="tp")
                for ti in range(TPC):
                    nc.tensor.transpose(
                        tp[:, ti * P:(ti + 1) * P],
                        z_tiles[ti][:, k * P:(k + 1) * P],
                        ident[:])
                zk = zntp.tile([P, TC], BF16, tag=f"znt{k}")
                nc.scalar.copy(zk[:], tp[:])
                znt.append(zk)

            # ---------- in-proj (v first: the scan chain hangs off it) ----
            for m in range(MT_V):    # v, full H
                ps = mmpool.tile([P, TC], F32, tag="mm")
                for k in range(KT):
                    nc.tensor.matmul(
                        ps[:], wv_sb[:, k * H + m * P: k * H + (m + 1) * P],
                        znt[k][:], start=(k == 0), stop=(k == KT - 1))
                ext = vpool.tile([P, TC + KCONV - 1], BF16, tag=f"v{m}")
                nc.scalar.add(ext[:, KCONV - 1:], ps[:], bv_sb[:, m:m + 1])
                if nb == 0:
                    nc.gpsimd.memset(ext[:, 0:KCONV - 1], 0.0)
                else:
                    nc.scalar.copy(ext[:, 0:KCONV - 1],
                                   prev_ext[m][:, TC:TC + KCONV - 1])

                # ---------- decay scan (chained across chunks) ----------
                st = spool.tile([P, TC], BF16, tag=f"s{m}")
                nc.vector.tensor_tensor_scan(
                    out=st[:],
                    data0=decay_sb[:, m:m + 1].broadcast_to([P, TC]),
                    data1=ext[:, KCONV - 1:],
                    initial=(0.0 if nb == 0 else state_sb[:, m:m + 1]),
                    op0=mybir.AluOpType.mult, op1=mybir.AluOpType.add)
                nc.scalar.copy(state_sb[:, m:m + 1], st[:, TC - 1:TC])
                s_tiles[m] = st
                prev_ext[m] = ext

            for mh in range(MT_H):   # gate half
                ps = mmpool.tile([P, TC], F32, tag="mm")
                for k in range(KT):
                    nc.tensor.matmul(
                        ps[:], wg_sb[:, k * HH + mh * P: k * HH + (mh + 1) * P],
                        znt[k][:], start=(k == 0), stop=(k == KT - 1))
                gt = gpool.tile([P, TC], BF16, tag=f"g{mh}")
                nc.scalar.activation(gt[:], ps[:],
                                     mybir.ActivationFunctionType.Sigmoid,
                                     bias=bg_sb[:, mh:mh + 1])
                g_tiles[mh] = gt

            # ---------- depthwise causal conv (local half) ----------
            dw_tiles = []
            for mh in range(MT_H):
                ext = prev_ext[mh]
                wof = mh * KCONV
                dw = dwpool.tile([P, TC], BF16, tag=f"dw{mh}")
                nc.vector.tensor_scalar_mul(dw[:], ext[:, 0:TC],
                                            dww_sb[:, wof:wof + 1])
                for j in range(1, KCONV):
                    nc.vector.scalar_tensor_tensor(
                        out=dw[:], in0=ext[:, j:j + TC],
                        scalar=dww_sb[:, wof + j:wof + j + 1],
                        in1=dw[:], op0=mybir.AluOpType.mult,
                        op1=mybir.AluOpType.add)
                dw_tiles.append(dw)

            # ---------- mix + v_new + gate ----------
            for mh in range(MT_H):
                ps = mmpool.tile([P, TC], F32, tag="mm")
                for k in range(MT_V):
                    nc.tensor.matmul(
                        ps[:], mixw_sb[:, k * HH + mh * P: k * HH + (mh + 1) * P],
                        s_tiles[k][:], start=(k == 0), stop=(k == MT_V - 1))
                # v_new = (mix + db) + dw   (in place into dw tile)
                nc.vector.scalar_tensor_tensor(
                    out=dw_tiles[mh][:], in0=ps[:],
                    scalar=db_sb[:, mh:mh + 1], in1=dw_tiles[mh][:],
                    op0=mybir.AluOpType.add, op1=mybir.AluOpType.add)
                # g *= v_new  (g tile becomes the out-proj lhsT)
                nc.vector.tensor_tensor(
                    out=g_tiles[mh][:], in0=g_tiles[mh][:], in1=dw_tiles[mh][:],
                    op=mybir.AluOpType.mult)

            # ---------- out-proj partial ----------
            for ti in range(TPC):
                for dc in range(2):
                    ps = popool.tile([P, 512], F32, tag="po")
                    for kh in range(MT_H):
                        nc.tensor.matmul(
                            ps[:], g_tiles[kh][:, ti * P:(ti + 1) * P],
                            outw_sb[:, kh * D + dc * 512: kh * D + (dc + 1) * 512],
                            start=(kh == 0), stop=(kh == MT_H - 1))
                    ysb = zpool.tile([P, 512], F32, tag="ysb")
                    nc.scalar.copy(ysb[:], ps[:])
                    t0 = nb * TC + ti * P
                    nc.sync.dma_start(y_d[t0:t0 + P, dc * 512:(dc + 1) * 512], ysb[:])


def _host_prep(inputs):
    x = np.asarray(inputs["x"], np.float32)
    gamma = np.asarray(inputs["norm_gamma"], np.float32)
    beta = np.asarray(inputs["norm_beta"], np.float32)
    in_w = np.asarray(inputs["in_w"], np.float32)
    in_b = np.asarray(inputs["in_b"], np.float32)
    dw_w = np.asarray(inputs["dw_w"], np.float32)
    dw_b = np.asarray(inputs["dw_b"], np.float32)
    log_decay = np.asarray(inputs["log_decay"], np.float32)
    mix_w = np.asarray(inputs["mix_w"], np.float32)
    mix_b = np.asarray(inputs["mix_b"], np.float32)
    out_w = np.asarray(inputs["out_w"], np.float32)

    w_fold = in_w * gamma[:, None]                     # fold gamma
    b_fold = beta @ in_w + in_b                        # fold beta
    decay = 1.0 / (1.0 + np.exp(-log_decay))
    db = dw_b + mix_b

    bf16 = ml_dtypes.bfloat16
    in_maps = []
    for c in range(8):
        b, j = divmod(c, 2)
        hs = j * HH
        # The device program always treats v-channel tiles 0..MT_H-1 as "the
        # local half" (depthwise conv, mix output, gating, out-proj).  Permute
        # the H axis per core so the local half comes first; everything that
        # indexes full H (wv columns, bv, decay, mixw rows) gets the same
        # permutation.
        perm = np.concatenate([np.arange(hs, hs + HH),
                               np.arange((1 - j) * HH, (1 - j) * HH + HH)])
        m = {
            "x": np.ascontiguousarray(x[b]),
            "wg": np.ascontiguousarray(w_fold[:, hs:hs + HH].astype(bf16)),
            "wv": np.ascontiguousarray(w_fold[:, H + perm].astype(bf16)),
            "mixw": np.ascontiguousarray(mix_w[perm][:, hs:hs + HH].astype(bf16)),
            "outw": np.ascontiguousarray(out_w[hs:hs + HH, :].astype(bf16)),
            "bg": np.ascontiguousarray(
                b_fold[hs:hs + HH].reshape(MT_H, P).T.astype(np.float32)),
            "bv": np.ascontiguousarray(
                b_fold[H + perm].reshape(MT_V, P).T.astype(np.float32)),
            "db": np.ascontiguousarray(
                db[hs:hs + HH].reshape(MT_H, P).T.astype(np.float32)),
            "decay": np.ascontiguousarray(
                decay[perm].reshape(MT_V, P).T.astype(np.float32)),
            "dww": np.ascontiguousarray(
                dw_w[hs:hs + HH].reshape(MT_H, P, KCONV)
                .transpose(1, 0, 2).reshape(P, MT_H * KCONV).astype(np.float32)),
        }
        in_maps.append(m)
    return in_maps


def get_nc():
    global _CACHED_NC
    if _CACHED_NC is None:
        _CACHED_NC = _build_core_program()
    return _CACHED_NC


_RUNNER = None


def _get_runner():
    """Build the 8-core shard_map runner once; repeated calls then skip
    re-tracing/compiling (run_bass_via_pjrt builds a fresh jit per call)."""
    global _RUNNER
    if _RUNNER is None:
        _RUNNER = make_runner(get_nc())
    return _RUNNER


def make_runner(nc, donate=True):
    import jax
    from jax.sharding import Mesh, PartitionSpec
    from jax.experimental.shard_map import shard_map
    import concourse.mybir as mb
    from concourse import bass2jax

    bass2jax.install_neuronx_cc_hook()

    partition_name = (nc.partition_id_tensor.name
                      if nc.partition_id_tensor else None)
    in_names, out_names, out_avals, zero_shapes = [], [], [], []
    for alloc in nc.m.functions[0].allocations:
        if not isinstance(alloc, mb.MemoryLocationSet):
            continue
        name = alloc.memorylocations[0].name
        if alloc.kind == "ExternalInput":
            if name != partition_name:
                in_names.append(name)
        elif alloc.kind == "ExternalOutput":
            out_names.append(name)
            shape = tuple(alloc.tensor_shape)
            dtype = mb.dt.np(alloc.dtype)
            out_avals.append(jax.core.ShapedArray(shape, dtype))
            zero_shapes.append((shape, dtype))
    n_params = len(in_names)
    all_names = in_names + out_names
    if partition_name is not None:
        all_names = all_names + [partition_name]
    donate = (tuple(range(n_params, n_params + len(out_names)))
              if donate else ())

    def _body(*args):
        operands = list(args)
        if partition_name is not None:
            operands.append(bass2jax.partition_id_tensor())
        outs = bass2jax._bass_exec_p.bind(
            *operands,
            out_avals=tuple(out_avals),
            in_names=tuple(all_names),
            out_names=tuple(out_names),
            lowering_input_output_aliases=(),
            sim_require_finite=True,
            sim_require_nnan=True,
            nc=nc,
        )
        return tuple(outs)

    devices = jax.devices()[:8]
    mesh = Mesh(np.asarray(devices), ("core",))
    nio = n_params + len(out_names)
    sharded = jax.jit(
        shard_map(_body, mesh=mesh,
                  in_specs=(PartitionSpec("core"),) * nio,
                  out_specs=(PartitionSpec("core"),) * len(out_names),
                  check_rep=False),
        donate_argnums=donate, keep_unused=True)
    return (sharded, in_names, out_names, out_avals, zero_shapes)


def _run_device(in_maps):
    sharded, in_names, out_names, out_avals, zero_shapes = _get_runner()
    concat_in = [
        np.concatenate([in_maps[c][n] for c in range(8)], axis=0)
        for n in in_names
    ]
    concat_zeros = [np.zeros((8 * s[0], *s[1:]), d) for s, d in zero_shapes]
    out_arrs = sharded(*concat_in, *concat_zeros)
    return [
        {n: np.asarray(out_arrs[i]).reshape(8, *out_avals[i].shape)[c]
         for i, n in enumerate(out_names)}
        for c in range(8)
    ]


def kernel(**inputs):
    in_maps = _host_prep(inputs)
    results = _run_device(in_maps)

    x = np.asarray(inputs["x"], np.float32)
    out_b = np.asarray(inputs["out_b"], np.float32)
    y = np.empty((BT, T, D), np.float32)
    for b in range(BT):
        y[b] = results[2 * b]["y"] + results[2 * b + 1]["y"]
    y += out_b
    y += x
    return y
